# revision 8
# baseline (speedup 1.0000x reference)
"""Trainium2 Bass kernel for nn_ConvGraphSelfLoop.

out = where(any(adj>=0, axes -1,-2), relu(features @ W + b), features)

Sharding: B*V = 65536 vertices split evenly across 8 NeuronCores (8192
each); W/bias replicated; no cross-core communication.

v2 design: fp8 DoubleRow matmuls with hi+lo error compensation.
  x ~= x_hi + x_lo  (fp8 e4m3 value + fp8 residual)
  W' = 32*W ~= w_hi + w_lo  (scaled so w_lo stays out of subnormals;
       the 1/32 is folded into the per-row mask scale at eviction)
  x@W' ~= x_hi@w_hi + x_lo@w_hi + x_hi@w_lo   (3 products, each at
       0.5 cycles/row via PerfMode.DoubleRow, K=256 per matmul)
Host pre-interleaves xT/W into the DoubleRow SBUF layout so every load
is one contiguous >=1KB-line DMA. Adjacency ships as int8 (adj>>24,
sign-preserving) in a tile-transposed layout, 8 tiles per DMA.

Per core, per 128-token tile:
  - PE: 4 big K=256 chunks x (hi@wh h0,h1; hi@wl h0,h1; lo@wh h0,h1)
        = 24 DoubleRow matmuls, N=512 each
  - ACT: r = relu(psum * mask/32) -> bf16
  - DVE: mask from adjacency; xc = x*(1-mask); out = r + xc
"""
import numpy as np
import ml_dtypes
import concourse.bass as bass
import concourse.bacc as bacc
import concourse.mybir as mybir
import concourse.tile as tile
from concourse.bass_utils import run_bass_kernel_spmd

B, V, E, NN = 4, 16384, 4, 32
F, U = 1024, 1024
ENN = E * NN
NCORES = 8
T = B * V // NCORES          # 8192 tokens per core
P = 128
NT = T // P                  # 64 token tiles
C2 = F // 256                # 4 double-row contraction chunks
NH = U // 512                # 2 u-halves
GK = 4                       # token tiles per xT DMA group
NG = NT // GK
SG = 8                       # token tiles per adjacency DMA group
WSCALE = 32.0

BF16 = ml_dtypes.bfloat16
FP8 = ml_dtypes.float8_e4m3

f32 = mybir.dt.float32
bf = mybir.dt.bfloat16
fp8 = mybir.dt.float8e4
i32 = mybir.dt.int32
i8 = mybir.dt.int8
AF = mybir.ActivationFunctionType
ALU = mybir.AluOpType
DR = mybir.MatmulPerfMode.DoubleRow

XT_COLS = T * F // P         # interleaved xT: [128, 65536] per core
GCOLS = C2 * 2 * GK * P      # 4096 cols per token-tile group


def _build(with_bias=False):
    nc = bacc.Bacc("TRN2", target_bir_lowering=False, debug=False,
                   num_devices=NCORES)
    xTh_d = nc.dram_tensor("featTh", [P, XT_COLS], fp8, kind="ExternalInput")
    xTl_d = nc.dram_tensor("featTl", [P, XT_COLS], fp8, kind="ExternalInput")
    x_d = nc.dram_tensor("feat", [T, F], bf, kind="ExternalInput")
    adj_d = nc.dram_tensor("adjacency", [P, NT * ENN], i8,
                           kind="ExternalInput")
    wh_d = nc.dram_tensor("wh", [P, C2 * 2 * U], fp8, kind="ExternalInput")
    wl_d = nc.dram_tensor("wl", [P, C2 * 2 * U], fp8, kind="ExternalInput")
    if with_bias:
        bias_d = nc.dram_tensor("bias", [1, U], bf, kind="ExternalInput")
    out_d = nc.dram_tensor("out", [T, U], bf, kind="ExternalOutput")

    with tile.TileContext(nc) as tc:
        with tc.tile_pool(name="const", bufs=1) as const, \
             tc.tile_pool(name="xth", bufs=2) as xth, \
             tc.tile_pool(name="xtl", bufs=2) as xtl, \
             tc.tile_pool(name="xp", bufs=4) as xp, \
             tc.tile_pool(name="adp", bufs=2) as adp, \
             tc.tile_pool(name="mp", bufs=4) as mp, \
             tc.tile_pool(name="rp", bufs=3) as rp, \
             tc.tile_pool(name="op", bufs=3) as op, \
             tc.tile_pool(name="psO", bufs=3, space="PSUM") as psO:

            # ---- startup constants ----
            # W chunks ride the (otherwise idle) GPSIMD DMA ring so they
            # don't serialize ahead of the first feature loads; c2-major
            # order so tile 0's c2=0 matmuls unblock first.
            wh_st = const.tile([P, C2 * 2 * U], fp8)
            wl_st = const.tile([P, C2 * 2 * U], fp8)
            for c2 in range(C2):
                cs = slice(c2 * 2 * U, (c2 + 1) * 2 * U)
                nc.gpsimd.dma_start(wh_st[:, cs], wh_d.ap()[:, cs])
                nc.gpsimd.dma_start(wl_st[:, cs], wl_d.ap()[:, cs])

            # PE warmup: walk the clock-ramp p-states during the initial
            # DMA fill so the real matmuls start at full rate.
            warm_l = const.tile([1, P], bf)
            nc.gpsimd.memset(warm_l[:], 0.0)
            warm_r = const.tile([1, 64], bf)
            nc.gpsimd.memset(warm_r[:], 0.0)
            with tc.tile_pool(name="psW", bufs=1, space="PSUM") as psW:
                wps = psW.tile([P, 64], f32)
                for _ in range(40):
                    nc.tensor.matmul(wps[:], warm_l[:], warm_r[:],
                                     start=True, stop=True)
            if with_bias:
                bias_st = const.tile([1, U], bf)
                nc.sync.dma_start(bias_st[:], bias_d.ap())
                ones_st = const.tile([1, P], f32)
                nc.gpsimd.memset(ones_st[:], 1.0)
                ones_b = const.tile([1, P], bf)
                nc.scalar.copy(ones_b[:], ones_st[:])

            adjg = None
            xgh = xgl = None
            for t in range(NT):
                if t % GK == 0:
                    g = t // GK
                    xgh = xth.tile([P, GCOLS], fp8, tag="xgh")
                    nc.sync.dma_start(
                        xgh[:], xTh_d.ap()[:, g * GCOLS:(g + 1) * GCOLS])
                    xgl = xtl.tile([P, GCOLS], fp8, tag="xgl")
                    nc.sync.dma_start(
                        xgl[:], xTl_d.ap()[:, g * GCOLS:(g + 1) * GCOLS])
                if t % SG == 0:
                    sg = t // SG
                    adjg = adp.tile([P, SG * ENN], i8, tag="adjg")
                    nc.sync.dma_start(
                        adjg[:], adj_d.ap()[:, sg * SG * ENN:
                                            (sg + 1) * SG * ENN])
                k = t % GK
                j = t % SG
                rows = slice(t * P, (t + 1) * P)

                # ---- DMA x (token-major, for the invalid-vertex path) ----
                x_t = xp.tile([P, F], bf, tag="x")
                nc.sync.dma_start(x_t[:], x_d.ap()[rows, :])

                # ---- DVE: mask pipeline ----
                mx = mp.tile([P, 1], i8, tag="mx")
                nc.vector.tensor_reduce(mx[:], adjg[:, j * ENN:(j + 1) * ENN],
                                        axis=mybir.AxisListType.X, op=ALU.max)
                # m_s = (mx >= 0) / WSCALE   (mask folded with 1/32)
                m_s = mp.tile([P, 1], f32, tag="m_s")
                nc.vector.tensor_scalar(m_s[:], mx[:], 0, 1.0 / WSCALE,
                                        ALU.is_ge, ALU.mult)
                minv = mp.tile([P, 1], f32, tag="minv")
                nc.vector.tensor_scalar(minv[:], mx[:], 0, None, ALU.is_lt)

                # ---- ACT: mask copy (washes DVE dep into ACT stream) ----
                m_act = mp.tile([P, 1], f32, tag="m_act")
                nc.scalar.copy(m_act[:], m_s[:])

                # ---- PE: 24 DoubleRow matmuls ----
                po = psO.tile([P, U], f32, tag="po")
                if with_bias:
                    for h in range(NH):
                        nc.tensor.matmul(po[:, h * 512:(h + 1) * 512],
                                         ones_b[:], bias_st[:, h * 512:
                                                            (h + 1) * 512],
                                         start=True, stop=False)
                for c2 in range(C2):
                    xb = c2 * (2 * GK * P) + k * (2 * P)
                    lhs_hi = xgh[:, xb:xb + 2 * P].rearrange(
                        "p (i m) -> p i m", i=2)
                    lhs_lo = xgl[:, xb:xb + 2 * P].rearrange(
                        "p (i m) -> p i m", i=2)
                    for lhs, w_st, last in ((lhs_hi, wh_st, False),
                                            (lhs_hi, wl_st, False),
                                            (lhs_lo, wh_st, True)):
                        for h in range(NH):
                            wb = c2 * (2 * U) + h * (2 * 512)
                            rhs = w_st[:, wb:wb + 2 * 512].rearrange(
                                "p (i n) -> p i n", i=2)
                            nc.tensor.matmul(
                                po[:, h * 512:(h + 1) * 512], lhs, rhs,
                                start=(c2 == 0 and lhs is lhs_hi
                                       and w_st is wh_st and not with_bias),
                                stop=(c2 == C2 - 1 and last),
                                perf_mode=DR)

                # ---- ACT: r = relu(psum * mask/32) ----
                r_t = rp.tile([P, U], bf, tag="r_t")
                nc.scalar.activation(r_t[:], po[:], AF.Relu, scale=m_act[:])

                # ---- DVE: xc = x*(1-mask); out = r + xc ----
                xc = xp.tile([P, F], bf, tag="xc")
                nc.vector.tensor_scalar(xc[:], x_t[:], minv[:], None,
                                        ALU.mult)
                out_t = op.tile([P, U], bf, tag="out_t")
                nc.vector.tensor_tensor(out=out_t[:], in0=r_t[:], in1=xc[:],
                                        op=ALU.add)

                # ---- DMA store (ACT ring: keeps loads and stores off
                # each other's queue; aggregate HBM has the headroom) ----
                nc.scalar.dma_start(out_d.ap()[rows, :], out_t[:])

    nc.compile()
    return nc


_nc_cache = {}


def _get_nc(with_bias=False):
    if with_bias not in _nc_cache:
        _nc_cache[with_bias] = _build(with_bias)
    return _nc_cache[with_bias]


def _interleave_xT(x32):
    """[T, F] fp32 -> (hi, lo) fp8 in the DoubleRow SBUF layout [128, T*F/128].

    col = g*(C2*2*GK*P) + c2*(2*GK*P) + k*(2*P) + i*P + m  maps to
    x[token = (g*GK + k)*P + m, f = c2*256 + i*128 + Ki(partition)]
    """
    hi = x32.astype(FP8)
    lo = (x32 - hi.astype(np.float32)).astype(FP8)
    out = []
    for a in (hi, lo):
        a6 = a.reshape(NG, GK, P, C2, 2, P)          # g k m c2 i Ki
        a6 = a6.transpose(5, 0, 3, 1, 4, 2)           # Ki g c2 k i m
        out.append(np.ascontiguousarray(a6.reshape(P, XT_COLS)))
    return out


def _interleave_w(w32):
    """[F, U] fp32*32 -> (hi, lo) fp8 [128, C2*2*U].

    col = c2*(2*U) + h*(2*512) + i*512 + n  maps to
    w[f = c2*256 + i*128 + Ki, u = h*512 + n]
    """
    w32 = w32 * WSCALE
    hi = w32.astype(FP8)
    lo = (w32 - hi.astype(np.float32)).astype(FP8)
    out = []
    for a in (hi, lo):
        a5 = a.reshape(C2, 2, P, NH, 512)             # c2 i Ki h n
        a5 = a5.transpose(2, 0, 3, 1, 4)              # Ki c2 h i n
        out.append(np.ascontiguousarray(a5.reshape(P, C2 * 2 * U)))
    return out


def _shard_inputs(inputs):
    feats = np.asarray(inputs["features"], dtype=np.float32).reshape(B * V, F)
    x16 = feats.astype(BF16)
    adj = np.asarray(inputs["adjacency"], dtype=np.int32).reshape(B * V, ENN)
    adj8 = (adj >> 24).astype(np.int8)   # sign-preserving downcast
    w32 = np.asarray(inputs["kernel"], dtype=np.float32)
    wh, wl = _interleave_w(w32)
    bias = np.asarray(inputs["bias"], dtype=np.float32).reshape(1, U)
    with_bias = bool(np.any(bias))
    in_maps = []
    for i in range(NCORES):
        s = slice(i * T, (i + 1) * T)
        xh, xl = _interleave_xT(feats[s])
        a3 = adj8[s].reshape(NT, P, ENN).transpose(1, 0, 2)  # p t e
        m = {
            "featTh": xh,
            "featTl": xl,
            "feat": x16[s],
            "adjacency": np.ascontiguousarray(a3.reshape(P, NT * ENN)),
            "wh": wh,
            "wl": wl,
        }
        if with_bias:
            m["bias"] = (bias * WSCALE).astype(BF16)
        in_maps.append(m)
    return in_maps


def _shard_expected(expected):
    e = expected.reshape(B * V, U)
    return [e[i * T:(i + 1) * T] for i in range(NCORES)]


def kernel(adjacency, features, kernel, bias):
    in_maps = _shard_inputs({"adjacency": adjacency, "features": features,
                             "kernel": kernel, "bias": bias})
    nc = _get_nc(with_bias="bias" in in_maps[0])
    res = run_bass_kernel_spmd(nc, in_maps, list(range(NCORES)))
    out = np.concatenate([res.results[i]["out"] for i in range(NCORES)],
                         axis=0)
    return out.reshape(B, V, U).astype(np.float32)
